# revision 1
# baseline (speedup 1.0000x reference)
"""DistanceNetwork (retrieval kNN cosine similarity) TRN2 Bass kernel.

reference:
    input_mag = rsqrt(max(sum(input**2), eps))              # global scalar
    support_mag = rsqrt(max(sum(support**2, axis=1), eps))  # [n]
    out[n, b, 0] = dot(support[n], input[b]) * support_mag[n] * input_mag

Shapes (hardcoded): support_set [8192, 1024] f32, input_image [2048, 1024] f32,
out [8192, 2048, 1] f32.

Sharding: support rows split across 8 cores (1024 rows / core); input_image
replicated (each core needs all of it for the global magnitude anyway, and
this halves HBM traffic vs replicating the 32MB support set). No collectives.

Device layout: host pre-transposes both operands so the contraction dim d
lands on SBUF partitions:
    s_t [1024 (d), 1024 (n_shard)]   x_t [1024 (d), 2048 (b)]
Main matmuls run in float32r (full PE rate, ~1.5e-4 scale-relative error).

Engines are strictly in-order, so emission order is chosen to match data
arrival: bt=0 matmuls stream kt-by-kt during the load, per-bt drains are
interleaved with per-bt x^2 squares on ACT, and the global input magnitude
uses GpSimd partition_all_reduce so it never enters the PE stream.
"""

import numpy as np

import concourse.bass as bass
import concourse.bacc as bacc
import concourse.bass_isa as bass_isa
import concourse.tile as tile
import concourse.mybir as mybir
from concourse.bass_utils import run_bass_kernel_spmd

F32 = mybir.dt.float32
F32R = mybir.dt.float32r
AF = mybir.ActivationFunctionType
ALU = mybir.AluOpType

D = 1024          # feature dim (contraction)
NS = 1024         # support rows per core
B = 2048          # query batch (replicated per core)
KT = D // 128     # 8 contraction tiles
NT = NS // 128    # 8 output-partition tiles
BT = B // 512     # 4 moving-dim chunks
EPS = 1e-10
N_CORES = 8


def _newton_rsqrt(nc, pool, a_ap, seed_ap, shape, pfx, iters=2):
    """r ~= rsqrt(a) refined from seed (1/sqrt via LUT) with Newton steps.

    r <- r * (1.5 - 0.5 * a * r * r).  All tiles [P, W] f32.
    """
    r = seed_ap
    for i in range(iters):
        t = pool.tile(shape, F32, tag=f"{pfx}_t{i}", name=f"{pfx}_t{i}")
        nc.vector.tensor_mul(t[:], r, r)
        nc.vector.tensor_mul(t[:], a_ap, t[:])
        nc.vector.tensor_scalar(
            t[:], t[:], -0.5, 1.5, op0=ALU.mult, op1=ALU.add
        )
        r2 = pool.tile(shape, F32, tag=f"{pfx}_r{i}", name=f"{pfx}_r{i}")
        nc.vector.tensor_mul(r2[:], r, t[:])
        r = r2[:]
    return r


def build_nc():
    nc = bacc.Bacc(None, target_bir_lowering=False)
    s_dram = nc.declare_dram_parameter("s_t", [D, NS], F32R, isOutput=False)
    x_dram = nc.declare_dram_parameter("x_t", [D, B], F32R, isOutput=False)
    o_dram = nc.declare_dram_parameter("out", [NS, B], F32, isOutput=True)
    ssq_dram = nc.dram_tensor("ssq_bounce", [NT, 128], F32)

    with tile.TileContext(nc) as tc:
        with (
            tc.tile_pool(name="sp", bufs=KT) as sp,
            tc.tile_pool(name="xp", bufs=KT * BT) as xp,
            tc.tile_pool(name="op", bufs=NT * BT) as op,
            tc.tile_pool(name="s2p", bufs=3) as s2p,
            tc.tile_pool(name="scrp", bufs=3) as scrp,
            tc.tile_pool(name="small", bufs=1) as small,
            tc.tile_pool(name="psum", bufs=8, space="PSUM") as psum,
        ):
            # ---- constants (tiny DMAs issued before the bulk loads) ---------
            ones = small.tile([128, 128], F32)
            nc.vector.memset(ones[:], 1.0)
            # pin ACT's sqrt table set before the Square stream starts, so the
            # mid-kernel Sqrt calls don't force a ~2.7us table reload
            ones_r = small.tile([128, 1], F32R)
            # f32r memset is invalid ISA; byte-copy 1.0f from the f32 ones
            nc.sync.dma_start(out=ones_r[:], in_=ones[:, 0:1].bitcast(F32R))
            sq_dummy = small.tile([1, 1], F32)
            nc.scalar.activation(sq_dummy[:], ones[0:1, 0:1], AF.Sqrt)

            accs = small.tile([128, KT * BT], F32)
            s_sb = [None] * KT
            s2_sb = [None] * KT
            x_sb = [[None] * BT for _ in range(KT)]
            o_sb = [[None] * NT for _ in range(BT)]

            def load_x(kt, bt):
                t = xp.tile([128, 512], F32R, tag="x_sb", name=f"x{kt}_{bt}")
                nc.sync.dma_start(
                    out=t[:],
                    in_=x_dram[kt * 128:(kt + 1) * 128, bt * 512:(bt + 1) * 512],
                )
                x_sb[kt][bt] = t

            def square_x(kt, bt):
                # per-partition sum of x^2 on ACT (TensorTensorReduce faults
                # on HW; ACT Square + free-dim accumulator works)
                scr = scrp.tile([128, 512], F32, tag="scr", name=f"scr{kt}_{bt}")
                nc.scalar.activation(
                    scr[:], x_sb[kt][bt][:].bitcast(F32), AF.Square,
                    accum_out=accs[:, (bt * KT + kt):(bt * KT + kt) + 1],
                )

            # ---- input DMAs: (x bt=0, s) interleaved in 128KB ring slices,
            # then x bt=1..3 ------------------------------------------------
            for kt in range(KT):
                load_x(kt, 0)
                t = sp.tile([128, NS], F32R, tag="s_sb", name=f"s{kt}")
                for q in range(2):
                    nc.sync.dma_start(
                        out=t[:, q * 512:(q + 1) * 512],
                        in_=s_dram[kt * 128:(kt + 1) * 128,
                                   q * 512:(q + 1) * 512],
                    )
                s_sb[kt] = t
                s2 = s2p.tile([128, NS], F32R, tag="s2", name=f"s2_{kt}")
                nc.vector.tensor_mul(s2[:], t[:], t[:])
                s2_sb[kt] = s2
                square_x(kt, 0)
            for bt in range(1, BT):
                for kt in range(KT):
                    load_x(kt, bt)

            def main_mm(ps_ap, kt, nt, bt):
                nc.tensor.matmul(
                    ps_ap,
                    s_sb[kt][:, nt * 128:(nt + 1) * 128],
                    x_sb[kt][bt][:],
                    start=(kt == 0),
                    stop=(kt == KT - 1),
                )

            def drain(bt, nt, ps_ap):
                # plain copy: PSUM frees at PE pace; both magnitude scales are
                # applied in the (already existing) second pass
                o = op.tile([128, 512], F32, tag="o", name=f"o{bt}_{nt}")
                nc.scalar.activation(o[:], ps_ap, AF.Copy)
                o_sb[bt][nt] = o

            # ---- bt=0: six nt-groups + the two ssq accumulators stream ------
            # kt-by-kt as each (s[kt], x[kt][0]) pair lands.  ssq uses the
            # ones COLUMN as the stationary operand (1-column weight load).
            ssq_ps = [
                psum.tile([1, 512], F32, tag="ps", name=f"ssq_ps{h}")
                for h in range(2)
            ]
            ps_g0 = [
                psum.tile([128, 512], F32, tag="ps", name=f"ps0_{nt}")
                for nt in range(6)
            ]
            for kt in range(KT):
                for nt in range(6):
                    main_mm(ps_g0[nt][:], kt, nt, 0)
                for h in range(2):
                    nc.tensor.matmul(
                        ssq_ps[h][:], ones_r[:],
                        s2_sb[kt][:, h * 512:(h + 1) * 512],
                        start=(kt == 0), stop=(kt == KT - 1),
                    )

            ssq_sb = small.tile([1, NS], F32)
            for h in range(2):
                nc.vector.tensor_copy(
                    ssq_sb[0:1, h * 512:(h + 1) * 512], ssq_ps[h][:]
                )
            str_sb = small.tile([128, NT], F32)

            for nt in range(6):
                drain(0, nt, ps_g0[nt][:])
            # bt=0 groups 6,7 run once the ssq accumulators free their banks
            ps_g67 = [
                psum.tile([128, 512], F32, tag="ps", name=f"ps0_{nt}")
                for nt in (6, 7)
            ]
            for i, nt in enumerate((6, 7)):
                for kt in range(KT):
                    main_mm(ps_g67[i][:], kt, nt, 0)
            for i, nt in enumerate((6, 7)):
                drain(0, nt, ps_g67[i][:])

            # ---- bt = 1..3 --------------------------------------------------
            comb = None
            for bt in range(1, BT):
                if bt == 1:
                    # ALL remaining squares + the full magnitude chain are
                    # emitted here, ahead of every PE-gated drain in the ACT
                    # stream -- so `comb` resolves as soon as the data
                    # arrives (~50us), not after the PE finishes bt2
                    for kt in range(KT):
                        square_x(kt, 1)
                    nc.scalar.dma_start(
                        out=ssq_dram[:],
                        in_=ssq_sb[0:1, :].rearrange("o (t p) -> o t p", p=128),
                    )
                    nc.scalar.dma_start(
                        out=str_sb[:], in_=ssq_dram.rearrange("t p -> p t")
                    )
                    for kt in range(KT):
                        square_x(kt, 2)
                    for kt in range(KT):
                        square_x(kt, 3)
                    smax = small.tile([128, NT], F32)
                    nc.vector.tensor_scalar_max(smax[:], str_sb[:], EPS)
                    s_sqrt = small.tile([128, NT], F32)
                    nc.scalar.activation(s_sqrt[:], smax[:], AF.Sqrt)
                    s_seed = small.tile([128, NT], F32)
                    nc.vector.reciprocal(s_seed[:], s_sqrt[:])
                    srs = _newton_rsqrt(
                        nc, small, smax[:], s_seed[:], [128, NT], "srs"
                    )
                    xsum = small.tile([128, 1], F32)
                    nc.vector.tensor_reduce(
                        xsum[:], accs[:], axis=mybir.AxisListType.X, op=ALU.add
                    )
                    xbc = small.tile([128, 1], F32)
                    nc.gpsimd.partition_all_reduce(
                        xbc[:], xsum[:], channels=128,
                        reduce_op=bass_isa.ReduceOp.add,
                    )
                    xmax = small.tile([128, 1], F32)
                    nc.vector.tensor_scalar_max(xmax[:], xbc[:], EPS)
                    x_sqrt = small.tile([128, 1], F32)
                    nc.scalar.activation(x_sqrt[:], xmax[:], AF.Sqrt)
                    x_seed = small.tile([128, 1], F32)
                    nc.vector.reciprocal(x_seed[:], x_sqrt[:])
                    xrs = _newton_rsqrt(
                        nc, small, xmax[:], x_seed[:], [128, 1], "xrs"
                    )
                    # combined per-(partition, nt) scale = support_mag * x_mag
                    comb = small.tile([128, NT], F32)
                    nc.vector.tensor_scalar(
                        comb[:], srs, xrs[:, 0:1], None, op0=ALU.mult
                    )
                ps_g = [
                    psum.tile([128, 512], F32, tag="ps", name=f"ps{bt}_{nt}")
                    for nt in range(NT)
                ]
                if bt == BT - 1:
                    for nt in range(NT):
                        for kt in range(KT):
                            main_mm(ps_g[nt][:], kt, nt, bt)
                else:
                    for kt in range(KT):
                        for nt in range(NT):
                            main_mm(ps_g[nt][:], kt, nt, bt)
                for nt in range(NT):
                    if bt >= 2:
                        # comb resolves (~52us) before these PE-gated drains
                        # execute: fuse the full scale and store directly
                        o = op.tile([128, 512], F32, tag="o", name=f"o{bt}_{nt}")
                        nc.scalar.activation(
                            o[:], ps_g[nt][:], AF.Copy, scale=comb[:, nt:nt + 1]
                        )
                        nc.sync.dma_start(
                            out=o_dram[nt * 128:(nt + 1) * 128,
                                       bt * 512:(bt + 1) * 512],
                            in_=o[:],
                        )
                    else:
                        drain(bt, nt, ps_g[nt][:])

            # ---- second pass: combined scale + store (bt 0..1) --------------
            for bt in range(2):
                for nt in range(NT):
                    o = o_sb[bt][nt]
                    nc.vector.tensor_scalar(
                        o[:], o[:], comb[:, nt:nt + 1], None, op0=ALU.mult
                    )
                    nc.sync.dma_start(
                        out=o_dram[nt * 128:(nt + 1) * 128, bt * 512:(bt + 1) * 512],
                        in_=o[:],
                    )
    nc.compile()
    return nc


_NC_CACHE = []


def _get_nc():
    if not _NC_CACHE:
        _NC_CACHE.append(build_nc())
    return _NC_CACHE[0]


def kernel(support_set: np.ndarray, input_image: np.ndarray) -> np.ndarray:
    support_set = np.asarray(support_set, dtype=np.float32)
    input_image = np.asarray(input_image, dtype=np.float32)
    assert support_set.shape == (N_CORES * NS, D)
    assert input_image.shape == (B, D)

    s_t = np.ascontiguousarray(support_set.T)  # [1024, 8192]
    x_t = np.ascontiguousarray(input_image.T)  # [1024, 2048]
    in_maps = [
        {
            "s_t": np.ascontiguousarray(s_t[:, i * NS:(i + 1) * NS]),
            "x_t": x_t,
        }
        for i in range(N_CORES)
    ]
    nc = _get_nc()
    res = run_bass_kernel_spmd(nc, in_maps, core_ids=list(range(N_CORES)))
    global LAST_RESULT
    LAST_RESULT = res
    out = np.concatenate([res.results[i]["out"] for i in range(N_CORES)], axis=0)
    return out[:, :, None]


LAST_RESULT = None



# revision 2
# speedup vs baseline: 1.1239x; 1.1239x over previous
"""DistanceNetwork (retrieval kNN cosine similarity) TRN2 Bass kernel.

reference:
    input_mag = rsqrt(max(sum(input**2), eps))              # global scalar
    support_mag = rsqrt(max(sum(support**2, axis=1), eps))  # [n]
    out[n, b, 0] = dot(support[n], input[b]) * support_mag[n] * input_mag

Shapes (hardcoded): support_set [8192, 1024] f32, input_image [2048, 1024] f32,
out [8192, 2048, 1] f32.

Sharding: support rows split across 8 cores (1024 rows / core); input_image
replicated. No collectives.

v2 design (vs the f32r baseline):
  * inputs cast to bf16 on host: input DMA drops 12.6MB -> 8MB and bf16
    weights legally pair with standalone LDWEIGHTS (f32r does not).
  * support is loaded TWICE: d-major s_t for the matmuls and row-major s_raw
    so the per-row sum-of-squares comes from ACT Square+accum directly in the
    [128, NT] layout the scale needs (kills the baseline's PE ones-matmuls
    and DRAM transpose bounce).  DMA has the slack; PE does not.
  * PSUM groups are {one 128-row support tile x all 4 batch chunks}, so each
    stationary tile is loaded once: post-compile surgery strips the
    sync-free duplicate LDWEIGHTS the compiler emits per matmul
    (272 loads -> 72).
  * drains/scales all run on DVE (reads PSUM directly), squares on ACT,
    partition reduce on GpSimd: PE stream is 256 matmuls + 72 weight loads.
  * stores are 1MB whole-row DMAs chased group-by-group; the last support
    tile is computed {bt0..2}+{bt3} so the final store tail is 1-2 tiles.
"""

import numpy as np
import ml_dtypes

import concourse.bass as bass
import concourse.bacc as bacc
import concourse.bass_isa as bass_isa
import concourse.tile as tile
import concourse.mybir as mybir
from concourse.bass_utils import run_bass_kernel_spmd

F32 = mybir.dt.float32
BF16 = mybir.dt.bfloat16
AF = mybir.ActivationFunctionType
ALU = mybir.AluOpType

D = 1024          # feature dim (contraction)
NS = 1024         # support rows per core
B = 2048          # query batch (replicated per core)
KT = D // 128     # 8 contraction tiles
NT = NS // 128    # 8 output-partition tiles
BT = B // 512     # 4 moving-dim chunks
EPS = 1e-10
N_CORES = 8


def _newton_rsqrt(nc, pool, a_ap, seed_ap, shape, pfx, iters=2):
    """r ~= rsqrt(a) refined from seed (1/sqrt via LUT) with Newton steps.

    r <- r * (1.5 - 0.5 * a * r * r).  All tiles [P, W] f32.
    """
    r = seed_ap
    for i in range(iters):
        t = pool.tile(shape, F32, tag=f"{pfx}_t{i}", name=f"{pfx}_t{i}")
        nc.vector.tensor_mul(t[:], r, r)
        nc.vector.tensor_mul(t[:], a_ap, t[:])
        nc.vector.tensor_scalar(
            t[:], t[:], -0.5, 1.5, op0=ALU.mult, op1=ALU.add
        )
        r2 = pool.tile(shape, F32, tag=f"{pfx}_r{i}", name=f"{pfx}_r{i}")
        nc.vector.tensor_mul(r2[:], r, t[:])
        r = r2[:]
    return r


def strip_dup_ldweights(nc):
    """Remove compiler-emitted LDWEIGHTS that reload the identical stationary
    AP already resident in the PE array.  Only sync-free duplicates are
    dropped, so the instruction removal carries no semaphore semantics."""
    removed = 0
    for f in nc.m.functions:
        for b in f.blocks:
            insts = b.instructions
            last_key = None
            to_remove = []
            for i in insts:
                tn = type(i).__name__
                if tn == 'InstLdweights':
                    ap = i.ins[0]
                    key = (ap.memref, ap.offset, str(ap.ap), str(ap.dtype),
                           str(i.perf_mode), str(i.is_transpose),
                           str(i.tile_position), str(i.tile_size))
                    si = i.sync_info
                    clean = (si is None) or (
                        len(si.on_wait) == 0 and len(si.on_update) == 0)
                    if key == last_key and clean:
                        to_remove.append(i)
                    else:
                        last_key = key
                elif tn in ('InstMatmult', 'InstMatmultMx'):
                    if getattr(i, 'is_transpose', False):
                        last_key = None
                elif tn in ('InstUnconditionalBranch', 'InstCompareBranch',
                            'InstCall'):
                    last_key = None
            for i in to_remove:
                insts.remove(i)
            removed += len(to_remove)
    return removed


def build_nc():
    nc = bacc.Bacc(None, target_bir_lowering=False)
    s_dram = nc.declare_dram_parameter("s_t", [D, NS], BF16, isOutput=False)
    x_dram = nc.declare_dram_parameter("x_t", [D, B], BF16, isOutput=False)
    sr_dram = nc.declare_dram_parameter("s_raw", [NS, D], BF16, isOutput=False)
    o_dram = nc.declare_dram_parameter("out", [NS, B], F32, isOutput=True)

    with tile.TileContext(nc) as tc:
        with (
            tc.tile_pool(name="xp", bufs=KT) as xp,
            tc.tile_pool(name="sp", bufs=KT) as sp,
            tc.tile_pool(name="srp", bufs=NT) as srp,
            tc.tile_pool(name="o01", bufs=2) as o01p,
            tc.tile_pool(name="og", bufs=3) as ogp,
            tc.tile_pool(name="ot", bufs=4) as otp,
            tc.tile_pool(name="sqx", bufs=2) as sqxp,
            tc.tile_pool(name="sqs", bufs=2) as sqsp,
            tc.tile_pool(name="small", bufs=1) as small,
            tc.tile_pool(name="psum", bufs=8, space="PSUM") as psum,
        ):
            x_sb = [None] * KT
            s_sb = [None] * KT
            sr_sb = [None] * NT

            # ---- bulk loads.  kt=0 is chunked so the PE's first matmul can
            # start after ~192KB instead of ~768KB.
            t = xp.tile([128, B], BF16, tag="x_sb", name="x0")
            x_sb[0] = t
            nc.sync.dma_start(out=t[:, 0:512], in_=x_dram[0:128, 0:512])
            t = sp.tile([128, NS], BF16, tag="s_sb", name="s0")
            s_sb[0] = t
            nc.sync.dma_start(out=t[:, 0:256], in_=s_dram[0:128, 0:256])
            for q in range(1, 4):
                nc.sync.dma_start(
                    out=x_sb[0][:, q * 512:(q + 1) * 512],
                    in_=x_dram[0:128, q * 512:(q + 1) * 512],
                )
            for kt in range(1, KT):
                t = xp.tile([128, B], BF16, tag="x_sb", name=f"x{kt}")
                nc.sync.dma_start(out=t[:], in_=x_dram[kt * 128:(kt + 1) * 128, :])
                x_sb[kt] = t
                t = sp.tile([128, NS], BF16, tag="s_sb", name=f"s{kt}")
                nc.sync.dma_start(
                    out=t[:, 0:256], in_=s_dram[kt * 128:(kt + 1) * 128, 0:256]
                )
                s_sb[kt] = t
            # stationary tails for groups nt=2..7, then the row-major copy
            for kt in range(KT):
                nc.sync.dma_start(
                    out=s_sb[kt][:, 256:NS],
                    in_=s_dram[kt * 128:(kt + 1) * 128, 256:NS],
                )
            for nt in range(NT):
                t = srp.tile([128, D], BF16, tag="sr_sb", name=f"sr{nt}")
                nc.sync.dma_start(out=t[:], in_=sr_dram[nt * 128:(nt + 1) * 128, :])
                sr_sb[nt] = t

            # ---- ACT: pin the sqrt table set before the Square stream so the
            # mid-kernel Sqrt calls don't force a ~1.3us table reload.
            ones = small.tile([1, 1], F32)
            nc.vector.memset(ones[:], 1.0)
            sq_dummy = small.tile([1, 1], F32)
            nc.scalar.activation(sq_dummy[:], ones[:], AF.Sqrt)

            # ---- ACT: sum-of-squares accumulators, in DMA arrival order
            xacc = small.tile([128, KT], F32)
            for kt in range(KT):
                scr = sqxp.tile([128, B], BF16, tag="sqx", name=f"sqx{kt}")
                nc.scalar.activation(
                    scr[:], x_sb[kt][:], AF.Square,
                    accum_out=xacc[:, kt:kt + 1],
                )
            sacc = small.tile([128, NT], F32)
            for nt in range(NT):
                scr = sqsp.tile([128, D], BF16, tag="sqs", name=f"sqs{nt}")
                nc.scalar.activation(
                    scr[:], sr_sb[nt][:], AF.Square,
                    accum_out=sacc[:, nt:nt + 1],
                )

            def mm(ps_ap, kt, nt, bt, start, stop):
                nc.tensor.matmul(
                    ps_ap,
                    s_sb[kt][:, nt * 128:(nt + 1) * 128],
                    x_sb[kt][:, bt * 512:(bt + 1) * 512],
                    start=start,
                    stop=stop,
                )

            # ---- PE phase A: groups nt=0,1 interleaved per kt so the PE pace
            # (~1.8us/kt) matches the load pace while x/s_t stream in.
            psA = [
                [psum.tile([128, 512], F32, tag="ps", name=f"psA{nt}_{bt}")
                 for bt in range(BT)]
                for nt in range(2)
            ]
            for kt in range(KT):
                for nt in range(2):
                    for bt in range(BT):
                        mm(psA[nt][bt][:], kt, nt, bt, kt == 0, kt == KT - 1)

            # ---- PE phase B: one support tile x all 4 batch chunks per
            # group; 4 banks/group ping-pong through the 8-buf pool.  The
            # last tile is {bt0..2}+{bt3} to shrink the final store tail.
            groupsB = [[(nt, (0, 1, 2, 3))] for nt in range(2, NT - 1)]
            groupsB.append([(NT - 1, (0, 1, 2))])
            groupsB.append([(NT - 1, (3,))])
            psB = []
            for gi, grp in enumerate(groupsB):
                tiles = {}
                for nt, bts in grp:
                    for bt in bts:
                        tiles[(nt, bt)] = psum.tile(
                            [128, 512], F32, tag="ps", name=f"psB{gi}_{nt}_{bt}"
                        )
                psB.append(tiles)
                for kt in range(KT):
                    for nt, bts in grp:
                        for bt in bts:
                            mm(tiles[(nt, bt)][:], kt, nt, bt,
                               kt == 0, kt == KT - 1)

            # ---- DVE: drain phase A unscaled (the combined scale is not
            # ready yet); rescaled in place + stored once `comb` resolves.
            oA = []
            for nt in range(2):
                o = o01p.tile([128, B], F32, tag="o01", name=f"oA{nt}")
                oA.append(o)
                for bt in range(BT):
                    nc.vector.tensor_copy(
                        o[:, bt * 512:(bt + 1) * 512], psA[nt][bt][:]
                    )

            # ---- magnitude chain
            xsum = small.tile([128, 1], F32)
            nc.vector.tensor_reduce(
                xsum[:], xacc[:], axis=mybir.AxisListType.X, op=ALU.add
            )
            xbc = small.tile([128, 1], F32)
            nc.gpsimd.partition_all_reduce(
                xbc[:], xsum[:], channels=128,
                reduce_op=bass_isa.ReduceOp.add,
            )
            xmax = small.tile([128, 1], F32)
            nc.vector.tensor_scalar_max(xmax[:], xbc[:], EPS)
            x_sqrt = small.tile([128, 1], F32)
            nc.scalar.activation(x_sqrt[:], xmax[:], AF.Sqrt)
            x_seed = small.tile([128, 1], F32)
            nc.vector.reciprocal(x_seed[:], x_sqrt[:])
            xrs = _newton_rsqrt(nc, small, xmax[:], x_seed[:], [128, 1], "xrs")

            smax = small.tile([128, NT], F32)
            nc.vector.tensor_scalar_max(smax[:], sacc[:], EPS)
            s_sqrt = small.tile([128, NT], F32)
            nc.scalar.activation(s_sqrt[:], smax[:], AF.Sqrt)
            s_seed = small.tile([128, NT], F32)
            nc.vector.reciprocal(s_seed[:], s_sqrt[:])
            srs = _newton_rsqrt(nc, small, smax[:], s_seed[:], [128, NT], "srs")

            # combined per-(partition, nt) scale = support_mag * input_mag
            comb = small.tile([128, NT], F32)
            nc.vector.tensor_scalar(
                comb[:], srs, xrs[:, 0:1], None, op0=ALU.mult
            )

            # ---- phase A rescale + store (1MB whole-row DMAs)
            for nt in range(2):
                nc.vector.tensor_scalar(
                    oA[nt][:], oA[nt][:], comb[:, nt:nt + 1], None, op0=ALU.mult
                )
                nc.sync.dma_start(
                    out=o_dram[nt * 128:(nt + 1) * 128, :], in_=oA[nt][:]
                )

            # ---- phase B: fused scale-drains on DVE straight from PSUM,
            # store per group
            for gi, grp in enumerate(groupsB):
                tiles = psB[gi]
                whole = len(grp) == 1 and len(grp[0][1]) == BT
                if whole:
                    nt = grp[0][0]
                    o = ogp.tile([128, B], F32, tag="og", name=f"oB{gi}")
                    for bt in range(BT):
                        nc.vector.tensor_scalar(
                            o[:, bt * 512:(bt + 1) * 512], tiles[(nt, bt)][:],
                            comb[:, nt:nt + 1], None, op0=ALU.mult,
                        )
                    nc.sync.dma_start(
                        out=o_dram[nt * 128:(nt + 1) * 128, :], in_=o[:]
                    )
                else:
                    for nt, bts in grp:
                        for bt in bts:
                            o = otp.tile([128, 512], F32, tag="ot",
                                         name=f"oT{gi}_{nt}_{bt}")
                            nc.vector.tensor_scalar(
                                o[:], tiles[(nt, bt)][:],
                                comb[:, nt:nt + 1], None, op0=ALU.mult,
                            )
                            nc.sync.dma_start(
                                out=o_dram[nt * 128:(nt + 1) * 128,
                                           bt * 512:(bt + 1) * 512],
                                in_=o[:],
                            )
    nc.compile()
    strip_dup_ldweights(nc)
    return nc


_NC_CACHE = []


def _get_nc():
    if not _NC_CACHE:
        _NC_CACHE.append(build_nc())
    return _NC_CACHE[0]


def kernel(support_set: np.ndarray, input_image: np.ndarray) -> np.ndarray:
    support_set = np.asarray(support_set, dtype=np.float32)
    input_image = np.asarray(input_image, dtype=np.float32)
    assert support_set.shape == (N_CORES * NS, D)
    assert input_image.shape == (B, D)

    bf16 = ml_dtypes.bfloat16
    x_t = np.ascontiguousarray(input_image.T).astype(bf16)  # [1024, 2048]
    in_maps = []
    for i in range(N_CORES):
        shard = support_set[i * NS:(i + 1) * NS]            # [1024, 1024]
        in_maps.append({
            "s_t": np.ascontiguousarray(shard.T).astype(bf16),
            "x_t": x_t,
            "s_raw": shard.astype(bf16),
        })
    nc = _get_nc()
    res = run_bass_kernel_spmd(nc, in_maps, core_ids=list(range(N_CORES)))
    global LAST_RESULT
    LAST_RESULT = res
    out = np.concatenate([res.results[i]["out"] for i in range(N_CORES)], axis=0)
    return out[:, :, None]


LAST_RESULT = None


# revision 3
# speedup vs baseline: 1.1731x; 1.0438x over previous
"""DistanceNetwork (retrieval kNN cosine similarity) TRN2 Bass kernel.

reference:
    input_mag = rsqrt(max(sum(input**2), eps))              # global scalar
    support_mag = rsqrt(max(sum(support**2, axis=1), eps))  # [n]
    out[n, b, 0] = dot(support[n], input[b]) * support_mag[n] * input_mag

Shapes (hardcoded): support_set [8192, 1024] f32, input_image [2048, 1024] f32,
out [8192, 2048, 1] f32.

Sharding: support rows split across 8 cores (1024 rows / core); input_image
replicated.  No collectives.

Design notes (v3):
  * inputs cast to bf16 on host: input DMA drops 12.6MB -> 8MB; bf16 matmul
    streams 1 column/cycle like f32r so PE time is unchanged.
  * support is loaded TWICE: d-major s_t for the matmuls and row-major s_raw
    so the per-row sum-of-squares comes from ACT Square+accum directly in the
    [128, NT] layout the scale needs (no PE ones-matmuls, no DRAM transpose
    bounce).  DMA has the slack; PE does not.
  * PSUM groups are {one 128-row support tile x all 4 batch chunks}, so each
    stationary tile loads once: post-compile surgery strips the sync-free
    duplicate LDWEIGHTS the compiler emits per matmul (272 -> ~72 loads).
  * load issue order: per-kt {x, s_t cols 0:512} (feeds groups nt0..3 and
    phase A), then s_raw (closes the magnitude chain by ~33us), then s_t
    cols 512:1024 (first needed ~41us).  The sync engine programs one
    descriptor per ~0.6us, so issue order IS arrival order.
  * DVE owns every PSUM drain (reads PSUM directly, fusing the combined
    scale); ACT owns the squares; GpSimd the partition reduce.  DVE order
    recycles G2/G3's banks before the phase-A rescale so the PE never waits
    on the scale chain.
  * 8 warm-up matmuls on a memset tile run during the engine preamble so the
    PE p-state ramp (~3us to full clock) finishes before real work arrives.
"""

import numpy as np
import ml_dtypes

import concourse.bass as bass
import concourse.bacc as bacc
import concourse.bass_isa as bass_isa
import concourse.tile as tile
import concourse.mybir as mybir
from concourse.bass_utils import run_bass_kernel_spmd

F32 = mybir.dt.float32
BF16 = mybir.dt.bfloat16
AF = mybir.ActivationFunctionType
ALU = mybir.AluOpType

D = 1024          # feature dim (contraction)
NS = 1024         # support rows per core
B = 2048          # query batch (replicated per core)
KT = D // 128     # 8 contraction tiles
NT = NS // 128    # 8 output-partition tiles
BT = B // 512     # 4 moving-dim chunks
EPS = 1e-10
N_CORES = 8
N_WARMUP = 8      # dummy matmuls to finish the PE p-state ramp pre-data


def _newton_rsqrt(nc, pool, a_ap, seed_ap, shape, pfx, iters=2):
    """r ~= rsqrt(a) refined from seed (1/sqrt via LUT) with Newton steps.

    r <- r * (1.5 - 0.5 * a * r * r).  All tiles [P, W] f32.
    """
    r = seed_ap
    for i in range(iters):
        t = pool.tile(shape, F32, tag=f"{pfx}_t{i}", name=f"{pfx}_t{i}")
        nc.vector.tensor_mul(t[:], r, r)
        nc.vector.tensor_mul(t[:], a_ap, t[:])
        nc.vector.tensor_scalar(
            t[:], t[:], -0.5, 1.5, op0=ALU.mult, op1=ALU.add
        )
        r2 = pool.tile(shape, F32, tag=f"{pfx}_r{i}", name=f"{pfx}_r{i}")
        nc.vector.tensor_mul(r2[:], r, t[:])
        r = r2[:]
    return r


def strip_dup_ldweights(nc):
    """Remove compiler-emitted LDWEIGHTS that reload the identical stationary
    AP already resident in the PE array.  Only sync-free duplicates are
    dropped, so removal carries no semaphore semantics."""
    removed = 0
    for f in nc.m.functions:
        for b in f.blocks:
            insts = b.instructions
            last_key = None
            to_remove = []
            for i in insts:
                tn = type(i).__name__
                if tn == 'InstLdweights':
                    ap = i.ins[0]
                    key = (ap.memref, ap.offset, str(ap.ap), str(ap.dtype),
                           str(i.perf_mode), str(i.is_transpose),
                           str(i.tile_position), str(i.tile_size))
                    si = i.sync_info
                    clean = (si is None) or (
                        len(si.on_wait) == 0 and len(si.on_update) == 0)
                    if key == last_key and clean:
                        to_remove.append(i)
                    else:
                        last_key = key
                elif tn in ('InstMatmult', 'InstMatmultMx'):
                    if getattr(i, 'is_transpose', False):
                        last_key = None
                elif tn in ('InstUnconditionalBranch', 'InstCompareBranch',
                            'InstCall'):
                    last_key = None
            for i in to_remove:
                insts.remove(i)
            removed += len(to_remove)
    return removed


def build_nc():
    nc = bacc.Bacc(None, target_bir_lowering=False)
    s_dram = nc.declare_dram_parameter("s_t", [D, NS], BF16, isOutput=False)
    x_dram = nc.declare_dram_parameter("x_t", [D, B], BF16, isOutput=False)
    sr_dram = nc.declare_dram_parameter("s_raw", [NS, D], BF16, isOutput=False)
    o_dram = nc.declare_dram_parameter("out", [NS, B], F32, isOutput=True)

    with tile.TileContext(nc) as tc:
        with (
            tc.tile_pool(name="xp", bufs=KT) as xp,
            tc.tile_pool(name="sp", bufs=KT) as sp,
            tc.tile_pool(name="srp", bufs=NT) as srp,
            tc.tile_pool(name="o01", bufs=2) as o01p,
            tc.tile_pool(name="og", bufs=3) as ogp,
            tc.tile_pool(name="ot", bufs=4) as otp,
            tc.tile_pool(name="sqx", bufs=2) as sqxp,
            tc.tile_pool(name="sqs", bufs=2) as sqsp,
            tc.tile_pool(name="small", bufs=1) as small,
            tc.tile_pool(name="psum", bufs=8, space="PSUM") as psum,
        ):
            x_sb = [None] * KT
            s_sb = [None] * KT
            sr_sb = [None] * NT

            # ---- bulk loads.  kt=0 is chunked so the PE's first matmul can
            # start after ~160KB instead of ~640KB.
            t = xp.tile([128, B], BF16, tag="x_sb", name="x0")
            x_sb[0] = t
            nc.sync.dma_start(out=t[:, 0:512], in_=x_dram[0:128, 0:512])
            t = sp.tile([128, NS], BF16, tag="s_sb", name="s0")
            s_sb[0] = t
            nc.sync.dma_start(out=t[:, 0:128], in_=s_dram[0:128, 0:128])
            for q in range(1, 4):
                nc.sync.dma_start(
                    out=x_sb[0][:, q * 512:(q + 1) * 512],
                    in_=x_dram[0:128, q * 512:(q + 1) * 512],
                )
            nc.sync.dma_start(out=s_sb[0][:, 128:512], in_=s_dram[0:128, 128:512])
            for kt in range(1, KT):
                t = xp.tile([128, B], BF16, tag="x_sb", name=f"x{kt}")
                nc.sync.dma_start(out=t[:], in_=x_dram[kt * 128:(kt + 1) * 128, :])
                x_sb[kt] = t
                t = sp.tile([128, NS], BF16, tag="s_sb", name=f"s{kt}")
                nc.sync.dma_start(
                    out=t[:, 0:512], in_=s_dram[kt * 128:(kt + 1) * 128, 0:512]
                )
                s_sb[kt] = t
            # row-major support next: it closes the magnitude chain, which
            # gates every store
            for nt in range(NT):
                t = srp.tile([128, D], BF16, tag="sr_sb", name=f"sr{nt}")
                nc.sync.dma_start(out=t[:], in_=sr_dram[nt * 128:(nt + 1) * 128, :])
                sr_sb[nt] = t
            # stationary tails for groups nt=4..7 (first needed ~41us)
            for kt in range(KT):
                nc.sync.dma_start(
                    out=s_sb[kt][:, 512:NS],
                    in_=s_dram[kt * 128:(kt + 1) * 128, 512:NS],
                )

            # ---- PE p-state warm-up: stream dummy matmuls on a memset tile
            # while the DMAs launch, so real work starts at full clock.  The
            # psum buf is recycled by the pool for phase A afterwards.
            wm_w = small.tile([128, 128], BF16)
            nc.vector.memset(wm_w[:], 0.0)
            wm_x = small.tile([128, 512], BF16)
            nc.vector.memset(wm_x[:], 0.0)
            ps_wm = psum.tile([128, 512], F32, tag="ps", name="ps_wm")
            for i in range(N_WARMUP):
                nc.tensor.matmul(ps_wm[:], wm_w[:], wm_x[:], start=True, stop=True)

            # ---- ACT: pin the sqrt table set before the Square stream so the
            # mid-kernel Sqrt calls don't force a ~1.3us table reload.
            ones = small.tile([1, 1], F32)
            nc.vector.memset(ones[:], 1.0)
            sq_dummy = small.tile([1, 1], F32)
            nc.scalar.activation(sq_dummy[:], ones[:], AF.Sqrt)

            # ---- ACT: sum-of-squares accumulators, in DMA arrival order
            xacc = small.tile([128, KT], F32)
            for kt in range(KT):
                scr = sqxp.tile([128, B], BF16, tag="sqx", name=f"sqx{kt}")
                nc.scalar.activation(
                    scr[:], x_sb[kt][:], AF.Square,
                    accum_out=xacc[:, kt:kt + 1],
                )
            sacc = small.tile([128, NT], F32)
            for nt in range(NT):
                scr = sqsp.tile([128, D], BF16, tag="sqs", name=f"sqs{nt}")
                nc.scalar.activation(
                    scr[:], sr_sb[nt][:], AF.Square,
                    accum_out=sacc[:, nt:nt + 1],
                )

            def mm(ps_ap, kt, nt, bt, start, stop):
                nc.tensor.matmul(
                    ps_ap,
                    s_sb[kt][:, nt * 128:(nt + 1) * 128],
                    x_sb[kt][:, bt * 512:(bt + 1) * 512],
                    start=start,
                    stop=stop,
                )

            # ---- PE phase A: groups nt=0,1 interleaved per kt so the PE pace
            # (~1.8us/kt) matches the load pace while x/s_t stream in.
            psA = [
                [psum.tile([128, 512], F32, tag="ps", name=f"psA{nt}_{bt}")
                 for bt in range(BT)]
                for nt in range(2)
            ]
            for kt in range(KT):
                for nt in range(2):
                    for bt in range(BT):
                        mm(psA[nt][bt][:], kt, nt, bt, kt == 0, kt == KT - 1)

            # ---- PE phase B: one support tile x all 4 batch chunks per
            # group; 4 banks/group ping-pong through the 8-buf pool.
            psB = {}
            for nt in range(2, NT):
                for bt in range(BT):
                    psB[(nt, bt)] = psum.tile(
                        [128, 512], F32, tag="ps", name=f"psB{nt}_{bt}"
                    )
                for kt in range(KT):
                    for bt in range(BT):
                        mm(psB[(nt, bt)][:], kt, nt, bt, kt == 0, kt == KT - 1)

            # ---- DVE: drain phase A unscaled (the combined scale is not
            # ready yet); rescaled in place + stored once `comb` resolves.
            oA = []
            for nt in range(2):
                o = o01p.tile([128, B], F32, tag="o01", name=f"oA{nt}")
                oA.append(o)
                for bt in range(BT):
                    nc.vector.tensor_copy(
                        o[:, bt * 512:(bt + 1) * 512], psA[nt][bt][:]
                    )

            # ---- magnitude chain
            xsum = small.tile([128, 1], F32)
            nc.vector.tensor_reduce(
                xsum[:], xacc[:], axis=mybir.AxisListType.X, op=ALU.add
            )
            xbc = small.tile([128, 1], F32)
            nc.gpsimd.partition_all_reduce(
                xbc[:], xsum[:], channels=128,
                reduce_op=bass_isa.ReduceOp.add,
            )
            xmax = small.tile([128, 1], F32)
            nc.vector.tensor_scalar_max(xmax[:], xbc[:], EPS)
            x_sqrt = small.tile([128, 1], F32)
            nc.scalar.activation(x_sqrt[:], xmax[:], AF.Sqrt)
            x_seed = small.tile([128, 1], F32)
            nc.vector.reciprocal(x_seed[:], x_sqrt[:])
            xrs = _newton_rsqrt(nc, small, xmax[:], x_seed[:], [128, 1], "xrs")

            smax = small.tile([128, NT], F32)
            nc.vector.tensor_scalar_max(smax[:], sacc[:], EPS)
            s_sqrt = small.tile([128, NT], F32)
            nc.scalar.activation(s_sqrt[:], smax[:], AF.Sqrt)
            s_seed = small.tile([128, NT], F32)
            nc.vector.reciprocal(s_seed[:], s_sqrt[:])
            srs = _newton_rsqrt(nc, small, smax[:], s_seed[:], [128, NT], "srs")

            # combined per-(partition, nt) scale = support_mag * input_mag
            comb = small.tile([128, NT], F32)
            nc.vector.tensor_scalar(
                comb[:], srs, xrs[:, 0:1], None, op0=ALU.mult
            )

            def drain_group(nt):
                """Fused scale-drain on DVE straight from PSUM + 1MB store."""
                o = ogp.tile([128, B], F32, tag="og", name=f"oB{nt}")
                for bt in range(BT):
                    nc.vector.tensor_scalar(
                        o[:, bt * 512:(bt + 1) * 512], psB[(nt, bt)][:],
                        comb[:, nt:nt + 1], None, op0=ALU.mult,
                    )
                nc.sync.dma_start(
                    out=o_dram[nt * 128:(nt + 1) * 128, :], in_=o[:]
                )

            # G2/G3 drains come FIRST on DVE so their banks recycle for G4/G5
            # without waiting on the phase-A rescale.
            drain_group(2)
            drain_group(3)

            # phase A rescale + store (1MB whole-row DMAs)
            for nt in range(2):
                nc.vector.tensor_scalar(
                    oA[nt][:], oA[nt][:], comb[:, nt:nt + 1], None, op0=ALU.mult
                )
                nc.sync.dma_start(
                    out=o_dram[nt * 128:(nt + 1) * 128, :], in_=oA[nt][:]
                )

            for nt in range(4, NT - 1):
                drain_group(nt)

            # last group: per-tile drains + stores to minimize the final tail
            nt = NT - 1
            for bt in range(BT):
                o = otp.tile([128, 512], F32, tag="ot", name=f"oT{bt}")
                nc.vector.tensor_scalar(
                    o[:], psB[(nt, bt)][:],
                    comb[:, nt:nt + 1], None, op0=ALU.mult,
                )
                nc.sync.dma_start(
                    out=o_dram[nt * 128:(nt + 1) * 128,
                               bt * 512:(bt + 1) * 512],
                    in_=o[:],
                )
    nc.compile()
    strip_dup_ldweights(nc)
    return nc


_NC_CACHE = []


def _get_nc():
    if not _NC_CACHE:
        _NC_CACHE.append(build_nc())
    return _NC_CACHE[0]


def kernel(support_set: np.ndarray, input_image: np.ndarray) -> np.ndarray:
    support_set = np.asarray(support_set, dtype=np.float32)
    input_image = np.asarray(input_image, dtype=np.float32)
    assert support_set.shape == (N_CORES * NS, D)
    assert input_image.shape == (B, D)

    bf16 = ml_dtypes.bfloat16
    x_t = np.ascontiguousarray(input_image.T).astype(bf16)  # [1024, 2048]
    in_maps = []
    for i in range(N_CORES):
        shard = support_set[i * NS:(i + 1) * NS]            # [1024, 1024]
        in_maps.append({
            "s_t": np.ascontiguousarray(shard.T).astype(bf16),
            "x_t": x_t,
            "s_raw": shard.astype(bf16),
        })
    nc = _get_nc()
    res = run_bass_kernel_spmd(nc, in_maps, core_ids=list(range(N_CORES)))
    global LAST_RESULT
    LAST_RESULT = res
    out = np.concatenate([res.results[i]["out"] for i in range(N_CORES)], axis=0)
    return out[:, :, None]


LAST_RESULT = None


# revision 10
# speedup vs baseline: 1.2096x; 1.0311x over previous
"""DistanceNetwork (retrieval kNN cosine similarity) TRN2 Bass kernel.

reference:
    input_mag = rsqrt(max(sum(input**2), eps))              # global scalar
    support_mag = rsqrt(max(sum(support**2, axis=1), eps))  # [n]
    out[n, b, 0] = dot(support[n], input[b]) * support_mag[n] * input_mag

Shapes (hardcoded): support_set [8192, 1024] f32, input_image [2048, 1024] f32,
out [8192, 2048, 1] f32.

Sharding: support rows split across 8 cores (1024 rows / core); input_image
replicated.  No collectives.

Design notes (v3):
  * inputs cast to bf16 on host: input DMA drops 12.6MB -> 8MB; bf16 matmul
    streams 1 column/cycle like f32r so PE time is unchanged.
  * support is loaded TWICE: d-major s_t for the matmuls and row-major s_raw
    so the per-row sum-of-squares comes from ACT Square+accum directly in the
    [128, NT] layout the scale needs (no PE ones-matmuls, no DRAM transpose
    bounce).  DMA has the slack; PE does not.
  * PSUM groups are {one 128-row support tile x all 4 batch chunks}, so each
    stationary tile loads once: post-compile surgery strips the sync-free
    duplicate LDWEIGHTS the compiler emits per matmul (272 -> ~72 loads).
  * load issue order: per-kt {x, s_t cols 0:512} (feeds groups nt0..3 and
    phase A), then s_raw (closes the magnitude chain by ~33us), then s_t
    cols 512:1024 (first needed ~41us).  The sync engine programs one
    descriptor per ~0.6us, so issue order IS arrival order.
  * DVE owns every PSUM drain (reads PSUM directly, fusing the combined
    scale); ACT owns the squares; GpSimd the partition reduce.  DVE order
    recycles G2/G3's banks before the phase-A rescale so the PE never waits
    on the scale chain.
  * 8 warm-up matmuls on a memset tile run during the engine preamble so the
    PE p-state ramp (~3us to full clock) finishes before real work arrives.
"""

import numpy as np
import ml_dtypes

import concourse.bass as bass
import concourse.bacc as bacc
import concourse.bass_isa as bass_isa
import concourse.tile as tile
import concourse.mybir as mybir
from concourse.bass_utils import run_bass_kernel_spmd

F32 = mybir.dt.float32
BF16 = mybir.dt.bfloat16
AF = mybir.ActivationFunctionType
ALU = mybir.AluOpType

D = 1024          # feature dim (contraction)
NS = 1024         # support rows per core
B = 2048          # query batch (replicated per core)
KT = D // 128     # 8 contraction tiles
NT = NS // 128    # 8 output-partition tiles
BT = B // 512     # 4 moving-dim chunks
EPS = 1e-10
N_CORES = 8
N_WARMUP = 8      # dummy matmuls to finish the PE p-state ramp pre-data


def _newton_rsqrt(nc, pool, a_ap, seed_ap, shape, pfx, iters=2):
    """r ~= rsqrt(a) refined from seed (1/sqrt via LUT) with Newton steps.

    r <- r * (1.5 - 0.5 * a * r * r).  All tiles [P, W] f32.
    """
    r = seed_ap
    for i in range(iters):
        t = pool.tile(shape, F32, tag=f"{pfx}_t{i}", name=f"{pfx}_t{i}")
        nc.vector.tensor_mul(t[:], r, r)
        nc.vector.tensor_mul(t[:], a_ap, t[:])
        nc.vector.tensor_scalar(
            t[:], t[:], -0.5, 1.5, op0=ALU.mult, op1=ALU.add
        )
        r2 = pool.tile(shape, F32, tag=f"{pfx}_r{i}", name=f"{pfx}_r{i}")
        nc.vector.tensor_mul(r2[:], r, t[:])
        r = r2[:]
    return r


def strip_dup_ldweights(nc):
    """Remove compiler-emitted LDWEIGHTS that reload the identical stationary
    AP already resident in the PE array.  Only sync-free duplicates are
    dropped, so removal carries no semaphore semantics."""
    removed = 0
    for f in nc.m.functions:
        for b in f.blocks:
            insts = b.instructions
            last_key = None
            to_remove = []
            for i in insts:
                tn = type(i).__name__
                if tn == 'InstLdweights':
                    ap = i.ins[0]
                    key = (ap.memref, ap.offset, str(ap.ap), str(ap.dtype),
                           str(i.perf_mode), str(i.is_transpose),
                           str(i.tile_position), str(i.tile_size))
                    si = i.sync_info
                    clean = (si is None) or (
                        len(si.on_wait) == 0 and len(si.on_update) == 0)
                    if key == last_key and clean:
                        to_remove.append(i)
                    else:
                        last_key = key
                elif tn in ('InstMatmult', 'InstMatmultMx'):
                    if getattr(i, 'is_transpose', False):
                        last_key = None
                elif tn in ('InstUnconditionalBranch', 'InstCompareBranch',
                            'InstCall'):
                    last_key = None
            for i in to_remove:
                insts.remove(i)
            removed += len(to_remove)
    return removed


def build_nc():
    nc = bacc.Bacc(None, target_bir_lowering=False)
    s_dram = nc.declare_dram_parameter("s_t", [D, NS], BF16, isOutput=False)
    x_dram = nc.declare_dram_parameter("x_t", [D, B], BF16, isOutput=False)
    sr_dram = nc.declare_dram_parameter("s_raw", [NS, D], BF16, isOutput=False)
    o_dram = nc.declare_dram_parameter("out", [NS, B], F32, isOutput=True)

    with tile.TileContext(nc) as tc:
        with (
            tc.tile_pool(name="xp", bufs=KT) as xp,
            tc.tile_pool(name="sp", bufs=KT) as sp,
            tc.tile_pool(name="srp", bufs=NT) as srp,
            tc.tile_pool(name="o01", bufs=2) as o01p,
            tc.tile_pool(name="og", bufs=3) as ogp,
            tc.tile_pool(name="ot", bufs=4) as otp,
            tc.tile_pool(name="sqx", bufs=2) as sqxp,
            tc.tile_pool(name="sqs", bufs=2) as sqsp,
            tc.tile_pool(name="small", bufs=1) as small,
            tc.tile_pool(name="psum", bufs=8, space="PSUM") as psum,
        ):
            x_sb = [None] * KT
            s_sb = [None] * KT
            sr_sb = [None] * NT

            # ---- bulk loads.  kt=0 is chunked so the PE's first matmul can
            # start after ~160KB instead of ~640KB.
            t = xp.tile([128, B], BF16, tag="x_sb", name="x0")
            x_sb[0] = t
            nc.sync.dma_start(out=t[:, 0:512], in_=x_dram[0:128, 0:512])
            t = sp.tile([128, NS], BF16, tag="s_sb", name="s0")
            s_sb[0] = t
            nc.sync.dma_start(out=t[:, 0:128], in_=s_dram[0:128, 0:128])
            for q in range(1, 4):
                nc.sync.dma_start(
                    out=x_sb[0][:, q * 512:(q + 1) * 512],
                    in_=x_dram[0:128, q * 512:(q + 1) * 512],
                )
            nc.sync.dma_start(out=s_sb[0][:, 128:256], in_=s_dram[0:128, 128:256])
            for kt in range(1, KT):
                t = xp.tile([128, B], BF16, tag="x_sb", name=f"x{kt}")
                nc.sync.dma_start(out=t[:], in_=x_dram[kt * 128:(kt + 1) * 128, :])
                x_sb[kt] = t
                t = sp.tile([128, NS], BF16, tag="s_sb", name=f"s{kt}")
                nc.sync.dma_start(
                    out=t[:, 0:256], in_=s_dram[kt * 128:(kt + 1) * 128, 0:256]
                )
                s_sb[kt] = t
            # stationary columns for G2/G3 (first needed as phase A ends)
            for kt in range(KT):
                nc.sync.dma_start(
                    out=s_sb[kt][:, 256:512],
                    in_=s_dram[kt * 128:(kt + 1) * 128, 256:512],
                )
            # row-major support next: it closes the magnitude chain, which
            # gates every store and the G2/G3 PSUM drains
            for nt in range(NT):
                t = srp.tile([128, D], BF16, tag="sr_sb", name=f"sr{nt}")
                nc.sync.dma_start(out=t[:], in_=sr_dram[nt * 128:(nt + 1) * 128, :])
                sr_sb[nt] = t
            # stationary tails for groups nt=4..7 (first needed ~40us)
            for kt in range(KT):
                nc.sync.dma_start(
                    out=s_sb[kt][:, 512:NS],
                    in_=s_dram[kt * 128:(kt + 1) * 128, 512:NS],
                )

            # ---- PE p-state warm-up: stream dummy matmuls on a memset tile
            # while the DMAs launch, so real work starts at full clock.  The
            # psum buf is recycled by the pool for phase A afterwards.
            wm_w = small.tile([128, 128], BF16)
            nc.vector.memset(wm_w[:], 0.0)
            wm_x = small.tile([128, 512], BF16)
            nc.vector.memset(wm_x[:], 0.0)
            ps_wm = psum.tile([128, 512], F32, tag="ps", name="ps_wm")
            for i in range(N_WARMUP):
                nc.tensor.matmul(ps_wm[:], wm_w[:], wm_x[:], start=True, stop=True)

            # ---- ACT: pin the sqrt table set before the Square stream so the
            # mid-kernel Sqrt calls don't force a ~1.3us table reload.
            ones = small.tile([1, 1], F32)
            nc.vector.memset(ones[:], 1.0)
            sq_dummy = small.tile([1, 1], F32)
            nc.scalar.activation(sq_dummy[:], ones[:], AF.Sqrt)

            # ---- ACT: sum-of-squares accumulators, in DMA arrival order
            xacc = small.tile([128, KT], F32)
            for kt in range(KT):
                scr = sqxp.tile([128, B], BF16, tag="sqx", name=f"sqx{kt}")
                nc.scalar.activation(
                    scr[:], x_sb[kt][:], AF.Square,
                    accum_out=xacc[:, kt:kt + 1],
                )
            sacc = small.tile([128, NT], F32)
            for nt in range(NT):
                scr = sqsp.tile([128, D], BF16, tag="sqs", name=f"sqs{nt}")
                nc.scalar.activation(
                    scr[:], sr_sb[nt][:], AF.Square,
                    accum_out=sacc[:, nt:nt + 1],
                )

            def mm(ps_ap, kt, nt, bt, start, stop):
                nc.tensor.matmul(
                    ps_ap,
                    s_sb[kt][:, nt * 128:(nt + 1) * 128],
                    x_sb[kt][:, bt * 512:(bt + 1) * 512],
                    start=start,
                    stop=stop,
                )

            # ---- PE phase A: groups nt=0,1 interleaved per kt so the PE pace
            # (~1.8us/kt) matches the load pace while x/s_t stream in.
            psA = [
                [psum.tile([128, 512], F32, tag="ps", name=f"psA{nt}_{bt}")
                 for bt in range(BT)]
                for nt in range(2)
            ]
            for kt in range(KT):
                for nt in range(2):
                    for bt in range(BT):
                        mm(psA[nt][bt][:], kt, nt, bt, kt == 0, kt == KT - 1)

            # ---- PE phase B: one support tile x all 4 batch chunks per
            # group; 4 banks/group ping-pong through the 8-buf pool.  The
            # final tile runs {bt0..2} then {bt3} so the last store tail is
            # one 256KB tile instead of four.
            grpsB = [(nt, (0, 1, 2, 3)) for nt in range(2, NT - 1)]
            grpsB.append((NT - 1, (0, 1, 2)))
            grpsB.append((NT - 1, (3,)))
            psB = {}
            for gi, (nt, bts) in enumerate(grpsB):
                tiles = {}
                for bt in bts:
                    tiles[bt] = psum.tile(
                        [128, 512], F32, tag="ps", name=f"psB{gi}_{nt}_{bt}"
                    )
                psB[gi] = tiles
                for kt in range(KT):
                    for bt in bts:
                        mm(tiles[bt][:], kt, nt, bt, kt == 0, kt == KT - 1)

            # ---- DVE: drain phase A unscaled (the combined scale is not
            # ready yet); rescaled in place + stored once `comb` resolves.
            oA = []
            for nt in range(2):
                o = o01p.tile([128, B], F32, tag="o01", name=f"oA{nt}")
                oA.append(o)
                for bt in range(BT):
                    nc.vector.tensor_copy(
                        o[:, bt * 512:(bt + 1) * 512], psA[nt][bt][:]
                    )

            # ---- magnitude chain
            xsum = small.tile([128, 1], F32)
            nc.vector.tensor_reduce(
                xsum[:], xacc[:], axis=mybir.AxisListType.X, op=ALU.add
            )
            xbc = small.tile([128, 1], F32)
            nc.gpsimd.partition_all_reduce(
                xbc[:], xsum[:], channels=128,
                reduce_op=bass_isa.ReduceOp.add,
            )
            xmax = small.tile([128, 1], F32)
            nc.vector.tensor_scalar_max(xmax[:], xbc[:], EPS)
            x_sqrt = small.tile([128, 1], F32)
            nc.scalar.activation(x_sqrt[:], xmax[:], AF.Sqrt)
            x_seed = small.tile([128, 1], F32)
            nc.vector.reciprocal(x_seed[:], x_sqrt[:])
            xrs = _newton_rsqrt(nc, small, xmax[:], x_seed[:], [128, 1], "xrs")

            smax = small.tile([128, NT], F32)
            nc.vector.tensor_scalar_max(smax[:], sacc[:], EPS)
            s_sqrt = small.tile([128, NT], F32)
            nc.scalar.activation(s_sqrt[:], smax[:], AF.Sqrt)
            s_seed = small.tile([128, NT], F32)
            nc.vector.reciprocal(s_seed[:], s_sqrt[:])
            srs = _newton_rsqrt(nc, small, smax[:], s_seed[:], [128, NT], "srs")

            # combined per-(partition, nt) scale = support_mag * input_mag
            comb = small.tile([128, NT], F32)
            nc.vector.tensor_scalar(
                comb[:], srs, xrs[:, 0:1], None, op0=ALU.mult
            )

            def drain_group(gi):
                """Fused scale-drain on DVE straight from PSUM + 1MB store."""
                nt = grpsB[gi][0]
                o = ogp.tile([128, B], F32, tag="og", name=f"oB{gi}")
                for bt in grpsB[gi][1]:
                    nc.vector.tensor_scalar(
                        o[:, bt * 512:(bt + 1) * 512], psB[gi][bt][:],
                        comb[:, nt:nt + 1], None, op0=ALU.mult,
                    )
                nc.sync.dma_start(
                    out=o_dram[nt * 128:(nt + 1) * 128, :], in_=o[:]
                )

            def drain_tiles(gi):
                """Per-tile drains + stores: shortest tail for final groups."""
                nt = grpsB[gi][0]
                for bt in grpsB[gi][1]:
                    o = otp.tile([128, 512], F32, tag="ot", name=f"oT{gi}_{bt}")
                    nc.vector.tensor_scalar(
                        o[:], psB[gi][bt][:],
                        comb[:, nt:nt + 1], None, op0=ALU.mult,
                    )
                    nc.sync.dma_start(
                        out=o_dram[nt * 128:(nt + 1) * 128,
                                   bt * 512:(bt + 1) * 512],
                        in_=o[:],
                    )

            # G2/G3 drains come FIRST on DVE so their banks recycle for G4/G5
            # without waiting on the phase-A rescale.
            drain_group(0)
            drain_group(1)

            # phase A rescale + store (1MB whole-row DMAs)
            for nt in range(2):
                nc.vector.tensor_scalar(
                    oA[nt][:], oA[nt][:], comb[:, nt:nt + 1], None, op0=ALU.mult
                )
                nc.sync.dma_start(
                    out=o_dram[nt * 128:(nt + 1) * 128, :], in_=oA[nt][:]
                )

            for gi in range(2, len(grpsB) - 2):
                drain_group(gi)
            drain_tiles(len(grpsB) - 2)
            drain_tiles(len(grpsB) - 1)
    nc.compile()
    strip_dup_ldweights(nc)
    return nc


_NC_CACHE = []


def _get_nc():
    if not _NC_CACHE:
        _NC_CACHE.append(build_nc())
    return _NC_CACHE[0]


def kernel(support_set: np.ndarray, input_image: np.ndarray) -> np.ndarray:
    support_set = np.asarray(support_set, dtype=np.float32)
    input_image = np.asarray(input_image, dtype=np.float32)
    assert support_set.shape == (N_CORES * NS, D)
    assert input_image.shape == (B, D)

    bf16 = ml_dtypes.bfloat16
    x_t = np.ascontiguousarray(input_image.T).astype(bf16)  # [1024, 2048]
    in_maps = []
    for i in range(N_CORES):
        shard = support_set[i * NS:(i + 1) * NS]            # [1024, 1024]
        in_maps.append({
            "s_t": np.ascontiguousarray(shard.T).astype(bf16),
            "x_t": x_t,
            "s_raw": shard.astype(bf16),
        })
    nc = _get_nc()
    res = run_bass_kernel_spmd(nc, in_maps, core_ids=list(range(N_CORES)))
    global LAST_RESULT
    LAST_RESULT = res
    out = np.concatenate([res.results[i]["out"] for i in range(N_CORES)], axis=0)
    return out[:, :, None]


LAST_RESULT = None
